# revision 27
# baseline (speedup 1.0000x reference)
"""MetaOptNet SVM-CS head on 8 Trainium2 NeuronCores.

Math: the reference runs a 15-iteration Mehrotra interior-point solve of the
Crammer-Singer dual QP per task. Empirically (f64 replication) the IPM is
fully converged by iteration 15, so the target equals the QP optimum. We
compute that optimum with a fixed-matrix ADMM:

    per task:  K = S S^T  (25x25 Gram)
               W~ = rho * (K + (1+rho) I)^{-1}   (Newton-Schulz: X1 analytic
                   = 2cI - c^2 H, one bf16 iteration, one fp32 polish;
                   |I - cH| <= ~0.1 since 9 <= eig(K+9I) <= ~17, and the
                   final fp32 iteration squares the bf16 error away)
               10x ADMM (rho=8), in (d1 = u-y, oy = y+oh/rho) state form:
                   t = center_ways(W~ @ d1) + oy
                   d1' = min(t, 2h - t);  oy' = max(t - (h - oh/rho), oh/rho)
                   where h = (C + 1/rho) oh
               compat = S Q^T  (25x75, bf16 inputs / f32 accum)
               logits = scale * compat^T @ x    (x = center_ways(W~ @ d1), f32)

The equality constraint A z = 0 (sum over ways per sample) reduces to
centering across ways because A A^T = n_way I; the KKT matrix is way-block-
diagonal with identical blocks K + (1+rho)I, which is what makes the single
25x25 inverse per task sufficient.

Sharding: pure data parallel, 16 tasks per core. Host-side work is layout
only (shard, transpose packing into 128-partition DMA tiles, one-hot
constants); all FLOPs run on-device.

I/O is deliberately minimal: the axon tunnel re-serializes every input
buffer on each execution (measured ~0.03 ms/MB of entropy + ~0.1 ms per
fragment), so the kernel ships exactly ONE tensor per core: a packed bf16
[128, 32032] holding support + query in d-major chunk layout, the one-hot
constants (0/0.125, bf16-exact), and scale as a bf16 (hi, lo) pair summed
to f32 on-device (~2^-16 relative). All other constants (identity
diagonals, h2/hmo scalings) are generated on-device with affine_select /
tensor_scalar. Support is shipped once; the second (sample-major) layout
the old kernel shipped for the w = S^T x stage is avoided by computing
compat = S Q^T instead and contracting logits = compat^T x over samples.

Precision: QP (Gram, inverse, ADMM) in fp32 with bf16 matmul inputs where
the error is quadratically damped; compat in bf16 inputs / f32 accumulate;
logits fully f32. Measured end-to-end ~2.9e-3 relative (tolerance 2e-2).

Tasks sit in 32-aligned 25-row partition blocks (PE tile_position
constraint), four tasks per 128-partition tile; zero padding rides through
every matmul/elementwise op harmlessly (the generated identities are full
128-diagonal; padding rows of the ADMM state stay exactly zero).
"""

import sys

sys.path.insert(0, "/opt/trn_rl_repo")

from contextlib import ExitStack

import numpy as np

import concourse.bass as bass
import concourse.tile as tile
from concourse import mybir
from concourse.alu_op_type import AluOpType
from concourse.bass_utils import run_bass_kernel_spmd
from concourse.tile import TileContext

# ---------------------------------------------------------------------------
# Problem constants (hardcoded per the harness contract)
N_CORES = 8
B_TOT = 128
T = 16            # tasks per core
NS = 25           # support samples per task
NW = 5            # ways
NQ = 75           # queries per task
D = 2560          # feature dim
NCH = D // 128    # 20 d-chunks
G = 4             # task groups per core (4 tasks each -> 100-partition tiles)
GP = T // G       # tasks per group
RHO = 8.0
NS_C = 0.065      # Newton-Schulz init scale for H = K + 9I
ADMM_ITERS = 10
C_REG = 0.1

ST_W = T * NS            # 400 cols per support chunk
QT_W = T * NQ            # 1200 cols per query chunk
ST_TOT = NCH * ST_W      # 8000
QT_TOT = NCH * QT_W      # 24000
AUX_O = ST_TOT + QT_TOT  # 32000: one-hot/rho (20 cols), scale hi/lo (2 cols)
BIG_W = AUX_O + 32       # 32032 (padded)

F32 = mybir.dt.float32
BF16 = mybir.dt.bfloat16


# ---------------------------------------------------------------------------
# The walrus build here encodes at most ONE sync-wait command per instruction
# (TPB_CTRL / S3_LW setupSyncWait raises "Too many sync wait commands").
# Tile's scheduler freely attaches several waits to one instruction, so after
# scheduling we split the excess onto NoOps inserted immediately before the
# instruction on the same engine — identical semantics, encodable waits.
def _split_waits(nc, max_waits=1):
    cnt = 0
    for blk in nc.m.functions[0].blocks:
        insns = blk.instructions
        idx = 0
        while idx < len(insns):
            ins = insns[idx]
            si = ins.sync_info
            waits = list(si.on_wait) if si and si.on_wait else []
            if len(waits) > max_waits:
                si.on_wait = waits[:max_waits]
                for w in waits[max_waits:]:
                    nop = mybir.InstNoOp(name=f"waitnop_{cnt}", ins=[], outs=[])
                    cnt += 1
                    nop.engine = ins.engine
                    nop.sync_info = mybir.SyncInfo(on_wait=[w], on_update=[])
                    nc.register_instruction(nop, overwrite=True)
                    insns.insert(idx, nop)
                    idx += 1
            idx += 1
    return cnt


# ---------------------------------------------------------------------------
def _build_program():
    nc = bass.Bass("TRN2", target_bir_lowering=False)

    big_d = nc.dram_tensor("big", [128, BIG_W], BF16, kind="ExternalInput")
    out_d = nc.dram_tensor("out", [NQ, T * NW], F32, kind="ExternalOutput")

    def st_ap(big_sb, c, t0, nt):
        o = c * ST_W + t0 * NS
        return big_sb[:, o : o + nt * NS]

    def qt_ap(big_sb, c, t0, nt):
        o = ST_TOT + c * QT_W + t0 * NQ
        return big_sb[:, o : o + nt * NQ]

    with ExitStack() as ctx:
        tc = ctx.enter_context(TileContext(nc))
        data = ctx.enter_context(tc.tile_pool(name="data", bufs=1))
        consts = ctx.enter_context(tc.tile_pool(name="consts", bufs=1))
        mats = ctx.enter_context(tc.tile_pool(name="mats", bufs=12))
        state = ctx.enter_context(tc.tile_pool(name="state", bufs=10))
        wout = ctx.enter_context(tc.tile_pool(name="wout", bufs=4))

        # ---- loads --------------------------------------------------------
        # Queue plan (cost model charges DMA transfer time to the issuing
        # engine's track; only sync/scalar/gpsimd can issue):
        #   gpsimd: aux tail + scale bcast FIRST (tiny, unblocks the ADMM
        #           constants), then query chunks 10-19
        #   sync:   support 0-9 (2 slices), then query chunks 0-9
        #   scalar: support 10-19 (2 slices); stays free for the NS-stage
        #           wt copies and the output activations afterwards
        big_sb = data.tile([128, BIG_W], BF16, tag="big")
        nc.gpsimd.dma_start(
            out=big_sb[:, AUX_O:BIG_W], in_=big_d[:, AUX_O:BIG_W]
        )
        # scale ships as a bf16 (hi, lo) pair; hi + lo in f32 recovers the
        # original f32 value to ~2^-16 relative
        schl_sb = consts.tile([NQ, 2], BF16, tag="schl")
        nc.gpsimd.dma_start(
            out=schl_sb, in_=big_d[0:1, AUX_O + 20 : AUX_O + 22].to_broadcast([NQ, 2])
        )
        hs = ST_TOT // 4
        for j in range(4):
            a = j * hs
            eng = nc.sync if j < 2 else nc.scalar
            eng.dma_start(out=big_sb[:, a : a + hs], in_=big_d[:, a : a + hs])

        # ---- on-device constants -----------------------------------------
        # Emitted BEFORE gpsimd's query DMAs: queue emission order is the
        # scheduler's priority, and these feed the Newton-Schulz stage at
        # ~7us -- they must not sit behind 9us of DMA on the Pool queue.
        # full 128-wide diagonals via affine_select (iota = p - f == 0)
        ones_sb = consts.tile([128, 128], F32, tag="ones")
        nc.vector.memset(ones_sb, 1.0)
        eye_sb = consts.tile([128, 128], F32, tag="eye")
        nc.gpsimd.affine_select(
            out=eye_sb,
            in_=ones_sb,
            pattern=[[-1, 128]],
            compare_op=AluOpType.is_equal,
            fill=0.0,
            base=0,
            channel_multiplier=1,
        )
        i2_sb = consts.tile([128, 128], F32, tag="i2")          # 2 I
        nc.vector.tensor_scalar_mul(i2_sb, eye_sb, 2.0)
        x1b_sb = consts.tile([128, 128], F32, tag="x1b")        # 2c I
        nc.gpsimd.tensor_scalar_mul(x1b_sb, eye_sb, 2.0 * NS_C)
        ohc_sb = consts.tile([128, 20], F32, tag="ohc")
        nc.vector.tensor_copy(ohc_sb, big_sb[:, AUX_O : AUX_O + 20])
        h2_sb = consts.tile([128, 20], F32, tag="h2")           # 2(C+1/rho) oh
        nc.gpsimd.tensor_scalar_mul(h2_sb, ohc_sb, 2.0 * (C_REG + 1.0 / RHO) * RHO)
        hmo_sb = consts.tile([128, 20], F32, tag="hmo")         # C oh
        nc.gpsimd.tensor_scalar_mul(hmo_sb, ohc_sb, C_REG * RHO)

        qh = QT_TOT // 4
        for j in range(4):
            a = ST_TOT + j * qh
            eng = nc.sync if j < 2 else nc.gpsimd
            eng.dma_start(out=big_sb[:, a : a + qh], in_=big_d[:, a : a + qh])

        # ---- stage 1: K = S S^T, block-diagonal per 4-task group ----------
        # h = K + (1+rho) I  built in one pass from the PSUM accumulate
        h_all = []
        with tc.tile_pool(name="kpsum", bufs=4, space="PSUM") as kpsum:
            for g in range(G):
                kp = kpsum.tile([128, 128], F32, tag="kp")
                nc.vector.memset(kp, 0.0)
                for c in range(NCH):
                    for tp in range(GP):
                        t = g * GP + tp
                        sl = slice(tp * 32, tp * 32 + NS)
                        nc.tensor.matmul(
                            kp[sl, sl],
                            lhsT=st_ap(big_sb, c, t, 1),
                            rhs=st_ap(big_sb, c, t, 1),
                            start=(c == 0),
                            stop=(c == NCH - 1),
                            tile_position=(0, tp * 32),
                        )
                h_sb = mats.tile([128, 128], F32, tag="h")
                nc.vector.scalar_tensor_tensor(
                    out=h_sb,
                    in0=i2_sb,
                    scalar=(1.0 + RHO) / 2.0,
                    in1=kp,
                    op0=AluOpType.mult,
                    op1=AluOpType.add,
                )
                h_all.append(h_sb)

        # ---- stage 2: Newton-Schulz inverse, 4 groups pipelined -----------
        # X1 = 2cI - c^2 H analytically (elementwise); one bf16 iteration
        # (NS self-corrects), final iter fp32 squares the bf16 error away,
        # so W~ is fp32-quality at a fraction of the PE cost.
        wt_sb = []
        with tc.tile_pool(name="npsum", bufs=4, space="PSUM") as npsum:
            for g in range(G):
                # X1 = 2cI - c^2 H, written straight to bf16 (one DVE op)
                x1b16 = mats.tile([128, 128], BF16, tag="x1b16")
                nc.vector.scalar_tensor_tensor(
                    out=x1b16,
                    in0=h_all[g],
                    scalar=-NS_C * NS_C,
                    in1=x1b_sb,
                    op0=AluOpType.mult,
                    op1=AluOpType.add,
                )
                hb = mats.tile([128, 128], BF16, tag="hb")
                nc.vector.tensor_copy(hb, h_all[g])
                # bf16 iteration: X2 = X1 (2I - H X1)
                t1p = npsum.tile([128, 128], F32, tag="t1p")
                nc.tensor.matmul(t1p, lhsT=hb, rhs=x1b16, start=True, stop=True)
                u1 = mats.tile([128, 128], BF16, tag="u1")
                nc.vector.tensor_tensor(u1, i2_sb, t1p, op=AluOpType.subtract)
                x2p = npsum.tile([128, 128], F32, tag="x2p")
                nc.tensor.matmul(x2p, lhsT=x1b16, rhs=u1, start=True, stop=True)
                x2f = mats.tile([128, 128], F32, tag="x2f")
                nc.vector.tensor_copy(x2f, x2p)
                # fp32 polish: X3 = X2 (2I - H X2);  W~ = rho X3
                t2p = npsum.tile([128, 128], F32, tag="t1p")
                nc.tensor.matmul(t2p, lhsT=h_all[g], rhs=x2f, start=True, stop=True)
                u2 = mats.tile([128, 128], F32, tag="u2")
                nc.vector.tensor_tensor(u2, i2_sb, t2p, op=AluOpType.subtract)
                x3p = npsum.tile([128, 128], F32, tag="x2p")
                nc.tensor.matmul(x3p, lhsT=x2f, rhs=u2, start=True, stop=True)
                wt = mats.tile([128, 128], F32, tag="wt")
                nc.scalar.activation(
                    wt, x3p, mybir.ActivationFunctionType.Copy, scale=RHO
                )
                wt_sb.append(wt)

        # ---- stages 3+4: ADMM (vector-bound) overlapped with compat (PE) --
        # t = center(Wt @ d1) + y + OHC;  d1' = min(t, 2h-t);
        # oy' = max(t - (h - OHC), OHC).  compat = S Q^T accumulates on the
        # tensor engine during the ADMM's vector-latency chain: the compat
        # matmul batches are emitted BEFORE each iteration's tiny matmuls so
        # the in-order PE queue fills ADMM stalls with compat work.
        d1_sb = ohc_sb
        oy_sb = ohc_sb
        xf_sb = None
        cp_sb = []
        with tc.tile_pool(name="cpsum", bufs=4, space="PSUM") as cpsum, \
             tc.tile_pool(name="mpsum", bufs=2, space="PSUM") as mpsum:
            cp_ps = []
            for g in range(G):
                cp_g = cpsum.tile([128, NQ], F32, tag="cp")
                cp_ps.append(cp_g)

            def compat_chunk(c):
                for g in range(G):
                    for tp in range(GP):
                        t = g * GP + tp
                        sl = slice(tp * 32, tp * 32 + NS)
                        nc.tensor.matmul(
                            cp_ps[g][sl, :],
                            lhsT=st_ap(big_sb, c, t, 1),
                            rhs=qt_ap(big_sb, c, t, 1),
                            start=(c == 0),
                            stop=(c == NCH - 1),
                            tile_position=(0, tp * 32),
                        )

            ci = 0
            for it in range(ADMM_ITERS):
                # two d-chunks of compat ahead of this iteration's matmuls
                for _ in range(2):
                    if ci < NCH:
                        compat_chunk(ci)
                        ci += 1
                xp = mpsum.tile([128, 20], F32, tag="mp")
                for g in range(G):
                    nc.tensor.matmul(
                        xp[:, g * NW : (g + 1) * NW],
                        lhsT=wt_sb[g],
                        rhs=d1_sb[:, g * NW : (g + 1) * NW],
                        start=True,
                        stop=True,
                    )
                msum = state.tile([128, 4], F32, tag="msum")
                nc.vector.reduce_sum(
                    msum,
                    xp[:, :].rearrange("p (g w) -> p g w", w=NW),
                    axis=mybir.AxisListType.X,
                )
                msb = msum[:, :]
                msb_ap = bass.AP(
                    tensor=msb.tensor,
                    offset=msb.offset,
                    ap=[msb.ap[0], msb.ap[1], [0, NW]],
                )
                p1 = state.tile([128, 20], F32, tag="p1")
                nc.vector.tensor_tensor(p1, xp, oy_sb, op=AluOpType.add)
                tt_sb = state.tile([128, 20], F32, tag="tt")
                nc.vector.scalar_tensor_tensor(
                    out=tt_sb[:, :].rearrange("p (g w) -> p g w", w=NW),
                    in0=msb_ap,
                    scalar=-1.0 / NW,
                    in1=p1[:, :].rearrange("p (g w) -> p g w", w=NW),
                    op0=AluOpType.mult,
                    op1=AluOpType.add,
                )
                if it == ADMM_ITERS - 1:
                    xf_sb = state.tile([128, 20], BF16, tag="xf")
                    nc.vector.scalar_tensor_tensor(
                        out=xf_sb[:, :].rearrange("p (g w) -> p g w", w=NW),
                        in0=msb_ap,
                        scalar=-1.0 / NW,
                        in1=xp[:, :].rearrange("p (g w) -> p g w", w=NW),
                        op0=AluOpType.mult,
                        op1=AluOpType.add,
                    )
                n2h = state.tile([128, 20], F32, tag="n2h")
                nc.vector.scalar_tensor_tensor(
                    out=n2h,
                    in0=tt_sb,
                    scalar=-1.0,
                    in1=h2_sb,
                    op0=AluOpType.mult,
                    op1=AluOpType.add,
                )
                d1_sb = state.tile([128, 20], F32, tag="d1n")
                nc.vector.tensor_tensor(d1_sb, tt_sb, n2h, op=AluOpType.min)
                if it < ADMM_ITERS - 1:
                    pa = state.tile([128, 20], F32, tag="pa")
                    nc.vector.tensor_tensor(pa, tt_sb, hmo_sb, op=AluOpType.subtract)
                    oy_sb = state.tile([128, 20], F32, tag="oy2")
                    nc.vector.tensor_tensor(oy_sb, pa, ohc_sb, op=AluOpType.max)
            while ci < NCH:
                compat_chunk(ci)
                ci += 1
            for g in range(G):
                cs = wout.tile([128, NQ], BF16, tag="cpsb")
                nc.vector.tensor_copy(cs, cp_ps[g])
                cp_sb.append(cs)

        # ---- stage 5: logits = scale * compat^T @ x (bf16 in, f32 psum) ---
        scale_sb = consts.tile([NQ, 1], F32, tag="scale")
        nc.vector.tensor_tensor(
            scale_sb, schl_sb[:, 0:1], schl_sb[:, 1:2], op=AluOpType.add
        )
        out_sb = consts.tile([NQ, T * NW], F32, tag="outsb")
        with tc.tile_pool(name="lpsum", bufs=3, space="PSUM") as lpsum:
            for t in range(T):
                g, tp = t // GP, t % GP
                psl = slice(tp * 32, tp * 32 + NS)
                lp = lpsum.tile([NQ, NW], F32, tag="lp")
                nc.tensor.matmul(
                    lp,
                    lhsT=cp_sb[g][psl, :],
                    rhs=xf_sb[psl, g * NW : (g + 1) * NW],
                    start=True,
                    stop=True,
                    tile_position=(tp * 32, 0),
                )
                nc.scalar.activation(
                    out_sb[:, t * NW : (t + 1) * NW],
                    lp,
                    mybir.ActivationFunctionType.Copy,
                    scale=scale_sb,
                )
        nc.sync.dma_start(out=out_d[:, :], in_=out_sb)

    _split_waits(nc)
    return nc


_NC_CACHE = None


def _get_nc():
    global _NC_CACHE
    if _NC_CACHE is None:
        _NC_CACHE = _build_program()
    return _NC_CACHE


# ---------------------------------------------------------------------------
def _host_prep(support, query, support_labels, scale):
    """Shard + pack into the DMA layouts. Layout only, no FLOPs."""
    f32 = np.float32
    bf = mybir.dt.np(BF16)
    sc = np.asarray(scale, dtype=f32).reshape(-1)

    in_maps = []
    for core in range(N_CORES):
        sl = slice(core * T, (core + 1) * T)
        S = np.asarray(support[sl], dtype=f32)        # [16,25,2560]
        Q = np.asarray(query[sl], dtype=f32)          # [16,75,2560]
        lab = np.asarray(support_labels[sl])          # [16,25] int
        st = S.transpose(2, 0, 1).reshape(NCH, 128, ST_W)
        qt = Q.transpose(2, 0, 1).reshape(NCH, 128, QT_W)
        big = np.zeros((128, BIG_W), dtype=bf)
        big[:, :ST_TOT] = np.moveaxis(st, 0, 1).reshape(128, ST_TOT).astype(bf)
        big[:, ST_TOT:AUX_O] = np.moveaxis(qt, 0, 1).reshape(128, QT_TOT).astype(bf)
        oh = (lab[:, :, None] == np.arange(NW)[None, None, :]).astype(f32)
        # [16,25,5] -> [100,20]: row = tp*32+s, col = g*5+w; values oh/rho
        # (0 or 0.125 -- exact in bf16)
        ohr = oh.reshape(G, GP, NS, NW).transpose(1, 2, 0, 3).reshape(GP, NS, 20)
        for tp in range(GP):
            big[tp * 32 : tp * 32 + NS, AUX_O : AUX_O + 20] = (ohr[tp] / RHO).astype(bf)
        hi = np.asarray(sc[0], dtype=bf)
        big[0, AUX_O + 20] = hi
        big[0, AUX_O + 21] = np.asarray(f32(sc[0]) - f32(hi), dtype=bf)
        in_maps.append({"big": np.ascontiguousarray(big)})
    return in_maps


def kernel(query, support, scale, support_labels, n_way, n_shot):
    assert int(n_way) == NW and int(n_shot) * int(n_way) == NS
    assert query.shape == (B_TOT, NQ, D) and support.shape == (B_TOT, NS, D)
    nc = _get_nc()
    in_maps = _host_prep(support, query, support_labels, scale)
    res = run_bass_kernel_spmd(nc, in_maps, core_ids=list(range(N_CORES)))
    outs = []
    for core in range(N_CORES):
        o = np.asarray(res.results[core]["out"])      # [75, 80]
        outs.append(o.reshape(NQ, T, NW).transpose(1, 0, 2))
    return np.ascontiguousarray(np.concatenate(outs, axis=0), dtype=np.float32)


# revision 32
# speedup vs baseline: 1.1614x; 1.1614x over previous
"""MetaOptNet SVM-CS head on 8 Trainium2 NeuronCores.

Math: the reference runs a 15-iteration Mehrotra interior-point solve of the
Crammer-Singer dual QP per task. Empirically (f64 replication) the IPM is
fully converged by iteration 15, so the target equals the QP optimum. We
compute that optimum with a fixed-matrix ADMM:

    per task:  K = S S^T  (25x25 Gram)
               W~ = rho * (K + (1+rho) I)^{-1}   (Newton-Schulz: X1 analytic
                   = 2cI - c^2 H, one bf16 iteration, one fp32 polish;
                   |I - cH| <= ~0.1 since 9 <= eig(K+9I) <= ~17, and the
                   final fp32 iteration squares the bf16 error away)
               10x ADMM (rho=8), in (d1 = u-y, oy = y+oh/rho) state form:
                   t = center_ways(W~ @ d1) + oy
                   d1' = min(t, 2h - t);  oy' = max(t - (h - oh/rho), oh/rho)
                   where h = (C + 1/rho) oh
               compat = S Q^T  (25x75, bf16 inputs / f32 accum)
               logits = scale * compat^T @ x    (x = center_ways(W~ @ d1), f32)

The equality constraint A z = 0 (sum over ways per sample) reduces to
centering across ways because A A^T = n_way I; the KKT matrix is way-block-
diagonal with identical blocks K + (1+rho)I, which is what makes the single
25x25 inverse per task sufficient.

Sharding: pure data parallel, 16 tasks per core. Host-side work is layout
only (shard, transpose packing into 128-partition DMA tiles, one-hot
constants); all FLOPs run on-device.

I/O is deliberately minimal: the axon tunnel re-serializes every input
buffer on each execution (measured ~0.03 ms/MB of entropy + ~0.1 ms per
fragment), so the kernel ships exactly ONE tensor per core: a packed bf16
[128, 32032] holding support + query in d-major chunk layout, the one-hot
constants (0/0.125, bf16-exact), and scale as a bf16 (hi, lo) pair summed
to f32 on-device (~2^-16 relative). All other constants (identity
diagonals, h2/hmo scalings) are generated on-device with affine_select /
tensor_scalar. Support is shipped once; the second (sample-major) layout
the old kernel shipped for the w = S^T x stage is avoided by computing
compat = S Q^T instead and contracting logits = compat^T x over samples.

Precision: QP (Gram, inverse, ADMM) in fp32 with bf16 matmul inputs where
the error is quadratically damped; compat in bf16 inputs / f32 accumulate,
then bf16 for the final logits contraction (linear error only). Measured
end-to-end ~3.7e-3 relative (tolerance 2e-2).

Device schedule (cost model 32.9us, was 38.4): support DMA split across the
sync+scalar queues and query across sync+gpsimd, with the tiny aux/constants
first on gpsimd so nothing latency-critical sits behind 9us of DMA on an
in-order queue; every matmul's consumers pay a ~1.3us PE pipeline-drain
latency, so the serial matmul->vector chains (Newton-Schulz, one hop per
ADMM iteration) set the floor, with compat filling the PE under the ADMM.

Tasks sit in 32-aligned 25-row partition blocks (PE tile_position
constraint), four tasks per 128-partition tile; zero padding rides through
every matmul/elementwise op harmlessly (the generated identities are full
128-diagonal; padding rows of the ADMM state stay exactly zero).
"""

import sys

sys.path.insert(0, "/opt/trn_rl_repo")

from contextlib import ExitStack

import numpy as np

import concourse.bass as bass
import concourse.tile as tile
from concourse import mybir
from concourse.alu_op_type import AluOpType
from concourse.bass_utils import run_bass_kernel_spmd
from concourse.tile import TileContext

# ---------------------------------------------------------------------------
# Problem constants (hardcoded per the harness contract)
N_CORES = 8
B_TOT = 128
T = 16            # tasks per core
NS = 25           # support samples per task
NW = 5            # ways
NQ = 75           # queries per task
D = 2560          # feature dim
NCH = D // 128    # 20 d-chunks
G = 4             # task groups per core (4 tasks each -> 100-partition tiles)
GP = T // G       # tasks per group
RHO = 8.0
NS_C = 0.065      # Newton-Schulz init scale for H = K + 9I
ADMM_ITERS = 10
C_REG = 0.1

ST_W = T * NS            # 400 cols per support chunk
QT_W = T * NQ            # 1200 cols per query chunk
ST_TOT = NCH * ST_W      # 8000
QT_TOT = NCH * QT_W      # 24000
AUX_O = ST_TOT + QT_TOT  # 32000: one-hot/rho (20 cols), scale hi/lo (2 cols)
BIG_W = AUX_O + 32       # 32032 (padded)

F32 = mybir.dt.float32
BF16 = mybir.dt.bfloat16


# ---------------------------------------------------------------------------
# The walrus build here encodes at most ONE sync-wait command per instruction
# (TPB_CTRL / S3_LW setupSyncWait raises "Too many sync wait commands").
# Tile's scheduler freely attaches several waits to one instruction, so after
# scheduling we split the excess onto NoOps inserted immediately before the
# instruction on the same engine — identical semantics, encodable waits.
def _split_waits(nc, max_waits=1):
    cnt = 0
    for blk in nc.m.functions[0].blocks:
        insns = blk.instructions
        idx = 0
        while idx < len(insns):
            ins = insns[idx]
            si = ins.sync_info
            waits = list(si.on_wait) if si and si.on_wait else []
            if len(waits) > max_waits:
                si.on_wait = waits[:max_waits]
                for w in waits[max_waits:]:
                    nop = mybir.InstNoOp(name=f"waitnop_{cnt}", ins=[], outs=[])
                    cnt += 1
                    nop.engine = ins.engine
                    nop.sync_info = mybir.SyncInfo(on_wait=[w], on_update=[])
                    nc.register_instruction(nop, overwrite=True)
                    insns.insert(idx, nop)
                    idx += 1
            idx += 1
    return cnt


# ---------------------------------------------------------------------------
def _build_program():
    nc = bass.Bass("TRN2", target_bir_lowering=False)

    big_d = nc.dram_tensor("big", [128, BIG_W], BF16, kind="ExternalInput")
    out_d = nc.dram_tensor("out", [NQ, T * NW], F32, kind="ExternalOutput")

    def st_ap(big_sb, c, t0, nt):
        o = c * ST_W + t0 * NS
        return big_sb[:, o : o + nt * NS]

    def qt_ap(big_sb, c, t0, nt):
        o = ST_TOT + c * QT_W + t0 * NQ
        return big_sb[:, o : o + nt * NQ]

    with ExitStack() as ctx:
        tc = ctx.enter_context(TileContext(nc))
        data = ctx.enter_context(tc.tile_pool(name="data", bufs=1))
        consts = ctx.enter_context(tc.tile_pool(name="consts", bufs=1))
        mats = ctx.enter_context(tc.tile_pool(name="mats", bufs=12))
        state = ctx.enter_context(tc.tile_pool(name="state", bufs=10))
        wout = ctx.enter_context(tc.tile_pool(name="wout", bufs=4))

        # ---- loads --------------------------------------------------------
        # Queue plan (cost model charges DMA transfer time to the issuing
        # engine's track; only sync/scalar/gpsimd can issue):
        #   gpsimd: aux tail + scale bcast FIRST (tiny, unblocks the ADMM
        #           constants), then query chunks 10-19
        #   sync:   support 0-9 (2 slices), then query chunks 0-9
        #   scalar: support 10-19 (2 slices); stays free for the NS-stage
        #           wt copies and the output activations afterwards
        big_sb = data.tile([128, BIG_W], BF16, tag="big")
        nc.gpsimd.dma_start(
            out=big_sb[:, AUX_O:BIG_W], in_=big_d[:, AUX_O:BIG_W]
        )
        # scale ships as a bf16 (hi, lo) pair; hi + lo in f32 recovers the
        # original f32 value to ~2^-16 relative
        schl_sb = consts.tile([NQ, 2], BF16, tag="schl")
        nc.gpsimd.dma_start(
            out=schl_sb, in_=big_d[0:1, AUX_O + 20 : AUX_O + 22].to_broadcast([NQ, 2])
        )
        hs = ST_TOT // 4
        for j in range(4):
            a = j * hs
            eng = nc.sync if j < 2 else nc.scalar
            eng.dma_start(out=big_sb[:, a : a + hs], in_=big_d[:, a : a + hs])

        # ---- on-device constants -----------------------------------------
        # Emitted BEFORE gpsimd's query DMAs: queue emission order is the
        # scheduler's priority, and these feed the Newton-Schulz stage at
        # ~7us -- they must not sit behind 9us of DMA on the Pool queue.
        # full 128-wide diagonals via affine_select (iota = p - f == 0)
        ones_sb = consts.tile([128, 128], F32, tag="ones")
        nc.vector.memset(ones_sb, 1.0)
        eye_sb = consts.tile([128, 128], F32, tag="eye")
        nc.gpsimd.affine_select(
            out=eye_sb,
            in_=ones_sb,
            pattern=[[-1, 128]],
            compare_op=AluOpType.is_equal,
            fill=0.0,
            base=0,
            channel_multiplier=1,
        )
        i2_sb = consts.tile([128, 128], F32, tag="i2")          # 2 I
        nc.vector.tensor_scalar_mul(i2_sb, eye_sb, 2.0)
        x1b_sb = consts.tile([128, 128], F32, tag="x1b")        # 2c I
        nc.gpsimd.tensor_scalar_mul(x1b_sb, eye_sb, 2.0 * NS_C)
        ohc_sb = consts.tile([128, 20], F32, tag="ohc")
        nc.vector.tensor_copy(ohc_sb, big_sb[:, AUX_O : AUX_O + 20])
        h2_sb = consts.tile([128, 20], F32, tag="h2")           # 2(C+1/rho) oh
        nc.gpsimd.tensor_scalar_mul(h2_sb, ohc_sb, 2.0 * (C_REG + 1.0 / RHO) * RHO)
        hmo_sb = consts.tile([128, 20], F32, tag="hmo")         # C oh
        nc.gpsimd.tensor_scalar_mul(hmo_sb, ohc_sb, C_REG * RHO)

        qh = QT_TOT // 4
        for j in range(4):
            a = ST_TOT + j * qh
            eng = nc.sync if j < 2 else nc.gpsimd
            eng.dma_start(out=big_sb[:, a : a + qh], in_=big_d[:, a : a + qh])

        # ---- stage 1: K = S S^T, block-diagonal per 4-task group ----------
        # h = K + (1+rho) I  built in one pass from the PSUM accumulate
        h_all = []
        with tc.tile_pool(name="kpsum", bufs=4, space="PSUM") as kpsum:
            for g in range(G):
                kp = kpsum.tile([128, 128], F32, tag="kp")
                nc.vector.memset(kp, 0.0)
                for c in range(NCH):
                    for tp in range(GP):
                        t = g * GP + tp
                        sl = slice(tp * 32, tp * 32 + NS)
                        nc.tensor.matmul(
                            kp[sl, sl],
                            lhsT=st_ap(big_sb, c, t, 1),
                            rhs=st_ap(big_sb, c, t, 1),
                            start=(c == 0),
                            stop=(c == NCH - 1),
                            tile_position=(0, tp * 32),
                        )
                h_sb = mats.tile([128, 128], F32, tag="h")
                nc.vector.scalar_tensor_tensor(
                    out=h_sb,
                    in0=i2_sb,
                    scalar=(1.0 + RHO) / 2.0,
                    in1=kp,
                    op0=AluOpType.mult,
                    op1=AluOpType.add,
                )
                h_all.append(h_sb)

        # ---- stage 2: Newton-Schulz inverse, 4 groups pipelined -----------
        # X1 = 2cI - c^2 H analytically (elementwise); one bf16 iteration
        # (NS self-corrects), final iter fp32 squares the bf16 error away,
        # so W~ is fp32-quality at a fraction of the PE cost.
        wt_sb = []
        with tc.tile_pool(name="npsum", bufs=4, space="PSUM") as npsum:
            for g in range(G):
                # X1 = 2cI - c^2 H, written straight to bf16 (one DVE op)
                x1b16 = mats.tile([128, 128], BF16, tag="x1b16")
                nc.vector.scalar_tensor_tensor(
                    out=x1b16,
                    in0=h_all[g],
                    scalar=-NS_C * NS_C,
                    in1=x1b_sb,
                    op0=AluOpType.mult,
                    op1=AluOpType.add,
                )
                hb = mats.tile([128, 128], BF16, tag="hb")
                nc.vector.tensor_copy(hb, h_all[g])
                # bf16 iteration: X2 = X1 (2I - H X1)
                t1p = npsum.tile([128, 128], F32, tag="t1p")
                nc.tensor.matmul(t1p, lhsT=hb, rhs=x1b16, start=True, stop=True)
                u1 = mats.tile([128, 128], BF16, tag="u1")
                nc.vector.tensor_tensor(u1, i2_sb, t1p, op=AluOpType.subtract)
                x2p = npsum.tile([128, 128], F32, tag="x2p")
                nc.tensor.matmul(x2p, lhsT=x1b16, rhs=u1, start=True, stop=True)
                x2f = mats.tile([128, 128], F32, tag="x2f")
                nc.vector.tensor_copy(x2f, x2p)
                # fp32 polish: X3 = X2 (2I - H X2);  W~ = rho X3
                t2p = npsum.tile([128, 128], F32, tag="t1p")
                nc.tensor.matmul(t2p, lhsT=h_all[g], rhs=x2f, start=True, stop=True)
                u2 = mats.tile([128, 128], F32, tag="u2")
                nc.vector.tensor_tensor(u2, i2_sb, t2p, op=AluOpType.subtract)
                x3p = npsum.tile([128, 128], F32, tag="x2p")
                nc.tensor.matmul(x3p, lhsT=x2f, rhs=u2, start=True, stop=True)
                wt = mats.tile([128, 128], F32, tag="wt")
                nc.scalar.activation(
                    wt, x3p, mybir.ActivationFunctionType.Copy, scale=RHO
                )
                wt_sb.append(wt)

        # ---- stages 3+4: ADMM (vector-bound) overlapped with compat (PE) --
        # t = center(Wt @ d1) + y + OHC;  d1' = min(t, 2h-t);
        # oy' = max(t - (h - OHC), OHC).  compat = S Q^T accumulates on the
        # tensor engine during the ADMM's vector-latency chain: the compat
        # matmul batches are emitted BEFORE each iteration's tiny matmuls so
        # the in-order PE queue fills ADMM stalls with compat work.
        d1_sb = ohc_sb
        oy_sb = ohc_sb
        xf_sb = None
        cp_sb = []
        with tc.tile_pool(name="cpsum", bufs=4, space="PSUM") as cpsum, \
             tc.tile_pool(name="mpsum", bufs=2, space="PSUM") as mpsum:
            cp_ps = []
            for g in range(G):
                cp_g = cpsum.tile([128, NQ], F32, tag="cp")
                cp_ps.append(cp_g)

            def compat_chunk(c):
                for g in range(G):
                    for tp in range(GP):
                        t = g * GP + tp
                        sl = slice(tp * 32, tp * 32 + NS)
                        nc.tensor.matmul(
                            cp_ps[g][sl, :],
                            lhsT=st_ap(big_sb, c, t, 1),
                            rhs=qt_ap(big_sb, c, t, 1),
                            start=(c == 0),
                            stop=(c == NCH - 1),
                            tile_position=(0, tp * 32),
                        )

            ci = 0
            for it in range(ADMM_ITERS):
                xp = mpsum.tile([128, 20], F32, tag="mp")
                for g in range(G):
                    nc.tensor.matmul(
                        xp[:, g * NW : (g + 1) * NW],
                        lhsT=wt_sb[g],
                        rhs=d1_sb[:, g * NW : (g + 1) * NW],
                        start=True,
                        stop=True,
                    )
                # two d-chunks of compat BEHIND this iteration's matmuls:
                # they fill the PE while the DVE chain runs, but can never
                # delay the next chain matmul ahead of them in the queue
                for _ in range(2):
                    if ci < NCH:
                        compat_chunk(ci)
                        ci += 1
                msum = state.tile([128, 4], F32, tag="msum")
                nc.vector.reduce_sum(
                    msum,
                    xp[:, :].rearrange("p (g w) -> p g w", w=NW),
                    axis=mybir.AxisListType.X,
                )
                msb = msum[:, :]
                msb_ap = bass.AP(
                    tensor=msb.tensor,
                    offset=msb.offset,
                    ap=[msb.ap[0], msb.ap[1], [0, NW]],
                )
                p1 = state.tile([128, 20], F32, tag="p1")
                nc.vector.tensor_tensor(p1, xp, oy_sb, op=AluOpType.add)
                tt_sb = state.tile([128, 20], F32, tag="tt")
                nc.vector.scalar_tensor_tensor(
                    out=tt_sb[:, :].rearrange("p (g w) -> p g w", w=NW),
                    in0=msb_ap,
                    scalar=-1.0 / NW,
                    in1=p1[:, :].rearrange("p (g w) -> p g w", w=NW),
                    op0=AluOpType.mult,
                    op1=AluOpType.add,
                )
                if it == ADMM_ITERS - 1:
                    xf_sb = state.tile([128, 20], BF16, tag="xf")
                    nc.vector.scalar_tensor_tensor(
                        out=xf_sb[:, :].rearrange("p (g w) -> p g w", w=NW),
                        in0=msb_ap,
                        scalar=-1.0 / NW,
                        in1=xp[:, :].rearrange("p (g w) -> p g w", w=NW),
                        op0=AluOpType.mult,
                        op1=AluOpType.add,
                    )
                n2h = state.tile([128, 20], F32, tag="n2h")
                nc.vector.scalar_tensor_tensor(
                    out=n2h,
                    in0=tt_sb,
                    scalar=-1.0,
                    in1=h2_sb,
                    op0=AluOpType.mult,
                    op1=AluOpType.add,
                )
                d1_sb = state.tile([128, 20], F32, tag="d1n")
                nc.vector.tensor_tensor(d1_sb, tt_sb, n2h, op=AluOpType.min)
                if it < ADMM_ITERS - 1:
                    pa = state.tile([128, 20], F32, tag="pa")
                    nc.vector.tensor_tensor(pa, tt_sb, hmo_sb, op=AluOpType.subtract)
                    oy_sb = state.tile([128, 20], F32, tag="oy2")
                    nc.vector.tensor_tensor(oy_sb, pa, ohc_sb, op=AluOpType.max)
            while ci < NCH:
                compat_chunk(ci)
                ci += 1
            for g in range(G):
                cs = wout.tile([128, NQ], BF16, tag="cpsb")
                nc.vector.tensor_copy(cs, cp_ps[g])
                cp_sb.append(cs)

        # ---- stage 5: logits = scale * compat^T @ x (bf16 in, f32 psum) ---
        scale_sb = consts.tile([NQ, 1], F32, tag="scale")
        nc.vector.tensor_tensor(
            scale_sb, schl_sb[:, 0:1], schl_sb[:, 1:2], op=AluOpType.add
        )
        out_sb = consts.tile([NQ, T * NW], F32, tag="outsb")
        ssb = scale_sb[:, :]
        scale_bc = bass.AP(tensor=ssb.tensor, offset=ssb.offset, ap=[ssb.ap[0], [0, NW]])
        with tc.tile_pool(name="lpsum", bufs=6, space="PSUM") as lpsum:
            for t in range(T):
                g, tp = t // GP, t % GP
                psl = slice(tp * 32, tp * 32 + NS)
                lp = lpsum.tile([NQ, NW], F32, tag="lp")
                nc.tensor.matmul(
                    lp,
                    lhsT=cp_sb[g][psl, :],
                    rhs=xf_sb[psl, g * NW : (g + 1) * NW],
                    start=True,
                    stop=True,
                    tile_position=(tp * 32, 0),
                )
                # scale+copy alternating DVE/scalar so the 16-task tail is
                # not serialized on one engine (each also releases lp)
                if t % 2 == 0:
                    nc.vector.tensor_tensor(
                        out_sb[:, t * NW : (t + 1) * NW],
                        lp,
                        scale_bc,
                        op=AluOpType.mult,
                    )
                else:
                    nc.scalar.activation(
                        out_sb[:, t * NW : (t + 1) * NW],
                        lp,
                        mybir.ActivationFunctionType.Copy,
                        scale=scale_sb,
                    )
        nc.sync.dma_start(out=out_d[:, :], in_=out_sb)

    _split_waits(nc)
    return nc


_NC_CACHE = None


def _get_nc():
    global _NC_CACHE
    if _NC_CACHE is None:
        _NC_CACHE = _build_program()
    return _NC_CACHE


# ---------------------------------------------------------------------------
def _host_prep(support, query, support_labels, scale):
    """Shard + pack into the DMA layouts. Layout only, no FLOPs."""
    f32 = np.float32
    bf = mybir.dt.np(BF16)
    sc = np.asarray(scale, dtype=f32).reshape(-1)

    in_maps = []
    for core in range(N_CORES):
        sl = slice(core * T, (core + 1) * T)
        S = np.asarray(support[sl], dtype=f32)        # [16,25,2560]
        Q = np.asarray(query[sl], dtype=f32)          # [16,75,2560]
        lab = np.asarray(support_labels[sl])          # [16,25] int
        st = S.transpose(2, 0, 1).reshape(NCH, 128, ST_W)
        qt = Q.transpose(2, 0, 1).reshape(NCH, 128, QT_W)
        big = np.zeros((128, BIG_W), dtype=bf)
        big[:, :ST_TOT] = np.moveaxis(st, 0, 1).reshape(128, ST_TOT).astype(bf)
        big[:, ST_TOT:AUX_O] = np.moveaxis(qt, 0, 1).reshape(128, QT_TOT).astype(bf)
        oh = (lab[:, :, None] == np.arange(NW)[None, None, :]).astype(f32)
        # [16,25,5] -> [100,20]: row = tp*32+s, col = g*5+w; values oh/rho
        # (0 or 0.125 -- exact in bf16)
        ohr = oh.reshape(G, GP, NS, NW).transpose(1, 2, 0, 3).reshape(GP, NS, 20)
        for tp in range(GP):
            big[tp * 32 : tp * 32 + NS, AUX_O : AUX_O + 20] = (ohr[tp] / RHO).astype(bf)
        hi = np.asarray(sc[0], dtype=bf)
        big[0, AUX_O + 20] = hi
        big[0, AUX_O + 21] = np.asarray(f32(sc[0]) - f32(hi), dtype=bf)
        in_maps.append({"big": np.ascontiguousarray(big)})
    return in_maps


def kernel(query, support, scale, support_labels, n_way, n_shot):
    assert int(n_way) == NW and int(n_shot) * int(n_way) == NS
    assert query.shape == (B_TOT, NQ, D) and support.shape == (B_TOT, NS, D)
    nc = _get_nc()
    in_maps = _host_prep(support, query, support_labels, scale)
    res = run_bass_kernel_spmd(nc, in_maps, core_ids=list(range(N_CORES)))
    outs = []
    for core in range(N_CORES):
        o = np.asarray(res.results[core]["out"])      # [75, 80]
        outs.append(o.reshape(NQ, T, NW).transpose(1, 0, 2))
    return np.ascontiguousarray(np.concatenate(outs, axis=0), dtype=np.float32)
